# revision 8
# baseline (speedup 1.0000x reference)
"""Trainium2 Bass kernel for MyMultiAttentionLayer.

Model (reference):
    q = einsum('bsd,hpd->bhsp', x, q_w) + q_b      (same for k, v)
    scores = q @ k^T / sqrt(P)                      [B,H,S,S]
    attn = softmax(scores, axis=2)                  # softmax over the QUERY axis
    ctx = einsum('bhqk,bhkp->bqhp', attn, v)
    out = concat(ctx) @ l_w.T + l_b                 [B,S,NUM_OUT]

Shapes: B=2, S=2048, D=1024, H=16, P=64, NUM_OUT=1024.

Sharding: 8 cores = 2 batches x 4 head-groups (4 heads each).  Each core
computes its batch's attention for its 4 heads plus the partial output
projection over its 256 features; the host sums the 4 partials per batch
(all-reduce equivalent) and adds l_b.

Key algebraic trick: softmax is over the query axis, so the normalizer
Z[k] = sum_q exp(s[q,k]) depends only on k.  ctx = sum_k e[q,k]*(v[k,:]/Z[k]),
i.e. the normalization can be folded into the tiny v rows (64 wide) instead
of the 2048-wide attention matrix.  exp() runs on the scalar engine with a
fused free-axis accumulate to produce Z in the same pass.

All matmuls use float32r (fp32 data, fast PE mode at moving dim >= 256).
float32r operands must be produced as float32r (DMA from f32r DRAM or
engine ops that round on write); fp32r matmul outputs cannot sit at a
PSUM partition offset, so every accumulator lives at partition base 0.

Per-core layouts (transposes are done host-side when staging inputs):
  xt  [D,S]   = x[b].T                    (contraction dim d on partitions)
  qwT [D,4P]  (d, (h,p))                  kwT same, vwT same
  qb  [4P,1]  kb [4P,1]  vb [1,4P]
  lwT [4P,NUM_OUT] = l_w[:, feat_slice].T
  out [S,NUM_OUT] partial (no l_b)
"""

import numpy as np

import concourse.bass as bass
import concourse.tile as tile
from concourse import bacc, mybir
from concourse.bass_utils import run_bass_kernel_spmd

B, S, D = 2, 2048, 1024
H, P = 16, 64
NUM_OUT = 1024
N_CORES = 8
HPC = 4                 # heads per core
PAIRS = 2               # head pairs per core (2 heads x 64 = 128 partitions)
DT = D // 128           # 8 d-tiles
ST = S // 128           # 16 s-tiles
SC = S // 512           # 4 s-chunks of 512
NC_CH = NUM_OUT // 512  # 2 output chunks

F32 = mybir.dt.float32
F32R = mybir.dt.float32r
EXP = mybir.ActivationFunctionType.Exp
AX = mybir.AxisListType.X


def build_nc():
    nc = bacc.Bacc("TRN2", target_bir_lowering=False, debug=False,
                   num_devices=N_CORES)

    xt_d = nc.dram_tensor("xt", [D, S], F32R, kind="ExternalInput")
    qwT_d = nc.dram_tensor("qwT", [D, HPC * P], F32R, kind="ExternalInput")
    kwT_d = nc.dram_tensor("kwT", [D, HPC * P], F32R, kind="ExternalInput")
    vwT_d = nc.dram_tensor("vwT", [D, HPC * P], F32R, kind="ExternalInput")
    qb_d = nc.dram_tensor("qb", [HPC * P, 1], F32, kind="ExternalInput")
    kb_d = nc.dram_tensor("kb", [HPC * P, 1], F32, kind="ExternalInput")
    vb_d = nc.dram_tensor("vb", [1, HPC * P], F32R, kind="ExternalInput")
    lwT_d = nc.dram_tensor("lwT", [HPC * P, NUM_OUT], F32R, kind="ExternalInput")
    ones_d = nc.dram_tensor("ones", [1, 128], F32R, kind="ExternalInput")
    out_d = nc.dram_tensor("out", [S, NUM_OUT], F32, kind="ExternalOutput")

    with tile.TileContext(nc) as tc:
        with (
            tc.tile_pool(name="qk", bufs=4) as p_qk,
            tc.tile_pool(name="vv", bufs=ST) as p_v,
            tc.tile_pool(name="cst", bufs=1) as p_c,
            tc.tile_pool(name="zz", bufs=4) as p_z,
            tc.tile_pool(name="mm", bufs=4, space=bass.MemorySpace.PSUM) as p_mm,
            tc.tile_pool(name="cx", bufs=4, space=bass.MemorySpace.PSUM) as p_cx,
        ):
            # ---- small constants ----
            qb_t, kb_t = [], []
            for pr in range(PAIRS):
                t = p_c.tile([128, 1], F32, name=f"qb{pr}", tag=f"qb{pr}")
                nc.sync.dma_start(t[:], qb_d[pr * 128:(pr + 1) * 128, :])
                qb_t.append(t)
                t = p_c.tile([128, 1], F32, name=f"kb{pr}", tag=f"kb{pr}")
                nc.sync.dma_start(t[:], kb_d[pr * 128:(pr + 1) * 128, :])
                kb_t.append(t)

            vb_t = p_c.tile([1, HPC * P], F32R, name="vb", tag="vb")
            nc.sync.dma_start(vb_t[:], vb_d[:, :])

            lw_t = []
            for h in range(HPC):
                t = p_c.tile([64, NUM_OUT], F32R, name=f"lw{h}", tag=f"lw{h}")
                nc.sync.dma_start(t[:], lwT_d[h * 64:(h + 1) * 64, :])
                lw_t.append(t)

            ones = p_c.tile([1, 128], F32R, name="ones", tag="ones")
            nc.sync.dma_start(ones[:], ones_d[:, :])

            # ---- projection phase (xt / weight staging scoped to it) ----
            qkT = {"q": [], "k": []}
            v_t = []
            with (
                tc.tile_pool(name="xt", bufs=DT) as p_xt,
                tc.tile_pool(name="wst", bufs=3 * DT) as p_w,
            ):
                xt = []
                for d in range(DT):
                    t = p_xt.tile([128, S], F32R, name=f"xt{d}", tag="xt")
                    nc.sync.dma_start(t[:], xt_d[d * 128:(d + 1) * 128, :])
                    xt.append(t)

                wt = {}
                for nm, dram in (("q", qwT_d), ("k", kwT_d), ("v", vwT_d)):
                    tiles = []
                    for d in range(DT):
                        t = p_w.tile([128, HPC * P], F32R, name=f"{nm}w{d}",
                                     tag="w")
                        nc.sync.dma_start(t[:], dram[d * 128:(d + 1) * 128, :])
                        tiles.append(t)
                    wt[nm] = tiles

                # q/k: qT/kT [128=(2 heads x P), S] per pair
                # out[p_hp, s] = sum_d wT[d, p_hp] * xt[d, s]
                for nm, bias in (("q", qb_t), ("k", kb_t)):
                    for pr in range(PAIRS):
                        ps = [p_mm.tile([128, 512], F32, name=f"ps_{nm}{pr}{c}",
                                        tag="mm") for c in range(SC)]
                        for d in range(DT):
                            lhsT = wt[nm][d][:, pr * 128:(pr + 1) * 128]
                            for c in range(SC):
                                nc.tensor.matmul(
                                    ps[c][:], lhsT,
                                    xt[d][:, c * 512:(c + 1) * 512],
                                    start=(d == 0), stop=(d == DT - 1))
                        dst = p_qk.tile([128, S], F32R, name=f"{nm}T{pr}",
                                        tag="qk")
                        for c in range(SC):
                            nc.vector.tensor_scalar_add(
                                dst[:, c * 512:(c + 1) * 512], ps[c][:],
                                bias[pr][:])
                        qkT[nm].append(dst)

                # v: [128=s, 4P=(h,p)] per s-tile; bias via ones-row matmul
                # out[s, hp] = sum_d xt[d, s] * vwT[d, hp]  (+ ones^T @ vb)
                for st in range(ST):
                    ps = p_mm.tile([128, 512], F32, name=f"ps_v{st}", tag="mm")
                    for d in range(DT):
                        nc.tensor.matmul(
                            ps[:, :HPC * P],
                            xt[d][:, st * 128:(st + 1) * 128],
                            wt["v"][d][:],
                            start=(d == 0), stop=False)
                    nc.tensor.matmul(ps[:, :HPC * P], ones[:], vb_t[:],
                                     start=False, stop=True)
                    dst = p_v.tile([128, HPC * P], F32R, name=f"v{st}", tag="v")
                    nc.vector.tensor_copy(dst[:], ps[:, :HPC * P])
                    v_t.append(dst)

            # ---- attention + output phase pools (reuse xt/wst space) ----
            attn_pools = (
                tc.tile_pool(name="et", bufs=8),
                tc.tile_pool(name="cc", bufs=HPC),
                tc.tile_pool(name="ob", bufs=2),
            )
            p_et = attn_pools[0].__enter__()
            p_cc = attn_pools[1].__enter__()
            p_ob = attn_pools[2].__enter__()

            # ---- attention, one head at a time ----
            # scoresT[k_i, q_i] = sum_p kT[p, k_i] * qT[p, q_i]   (K=64)
            # eT = exp(scoresT/8) with fused Z = sum_q (scalar engine)
            # v' = v * (1/Z) per k row; ctxT[p, q_i] += vs^T-stationary @ eT
            ctxT = []
            for h in range(HPC):
                pr, off = divmod(h, 2)
                off *= 64
                kT, qT = qkT["k"][pr], qkT["q"][pr]
                cps = [p_cx.tile([64, 512], F32, name=f"cx{h}{c}", tag="cx")
                       for c in range(SC)]
                for t in range(ST):
                    zp = p_z.tile([128, SC], F32, name=f"zp{h}{t}", tag="zp")
                    ets = []
                    for c in range(SC):
                        ps = p_mm.tile([128, 512], F32, name=f"ps_s{h}{t}{c}",
                                       tag="mm")
                        nc.tensor.matmul(
                            ps[:],
                            kT[off:off + 64, t * 128:(t + 1) * 128],
                            qT[off:off + 64, c * 512:(c + 1) * 512],
                            start=True, stop=True)
                        et = p_et.tile([128, 512], F32R, name=f"et{h}{t}{c}",
                                       tag="et")
                        nc.scalar.activation(et[:], ps[:], EXP, scale=0.125,
                                             accum_out=zp[:, c:c + 1])
                        ets.append(et)
                    z = p_z.tile([128, 1], F32, name=f"z{h}{t}", tag="z")
                    nc.vector.reduce_sum(z[:], zp[:], axis=AX)
                    zr = p_z.tile([128, 1], F32, name=f"zr{h}{t}", tag="zr")
                    nc.vector.reciprocal(zr[:], z[:])
                    vs = p_z.tile([128, 64], F32R, name=f"vs{h}{t}", tag="vs")
                    nc.vector.tensor_scalar_mul(
                        vs[:], v_t[t][:, h * 64:(h + 1) * 64], zr[:])
                    for c in range(SC):
                        nc.tensor.matmul(cps[c][:], vs[:], ets[c][:],
                                         start=(t == 0), stop=(t == ST - 1))
                dst = p_cc.tile([64, S], F32R, name=f"ctxT{h}", tag="cc")
                for c in range(SC):
                    nc.vector.tensor_copy(dst[:, c * 512:(c + 1) * 512],
                                          cps[c][:])
                ctxT.append(dst)

            # ---- output projection ----
            # out[s, n] = sum_h sum_p ctxT_h[p, s] * lwT_h[p, n]
            for st in range(ST):
                ob = p_ob.tile([128, NUM_OUT], F32, name=f"ob{st}", tag="ob")
                for ncn in range(NC_CH):
                    ps = p_mm.tile([128, 512], F32, name=f"ps_o{st}{ncn}",
                                   tag="mm")
                    for h in range(HPC):
                        nc.tensor.matmul(
                            ps[:],
                            ctxT[h][:, st * 128:(st + 1) * 128],
                            lw_t[h][:, ncn * 512:(ncn + 1) * 512],
                            start=(h == 0), stop=(h == HPC - 1))
                    nc.vector.tensor_copy(ob[:, ncn * 512:(ncn + 1) * 512],
                                          ps[:])
                nc.sync.dma_start(out_d[st * 128:(st + 1) * 128, :], ob[:])

            for cmgr in reversed(attn_pools):
                cmgr.__exit__(None, None, None)

    nc.compile()
    return nc


_NC_CACHE = None


def _get_nc():
    global _NC_CACHE
    if _NC_CACHE is None:
        _NC_CACHE = build_nc()
    return _NC_CACHE


def _prep_in_maps(x, q_w, q_b, k_w, k_b, v_w, v_b, l_w):
    """Host-side sharding: per-core input dict (core = b*4 + g)."""
    in_maps = []
    xts = [np.ascontiguousarray(x[b].T) for b in range(B)]
    ones = np.ones((1, 128), dtype=np.float32)
    for b in range(B):
        for g in range(4):
            hs = slice(g * HPC, (g + 1) * HPC)
            f0, f1 = g * HPC * P, (g + 1) * HPC * P
            in_maps.append({
                "xt": xts[b],
                "qwT": np.ascontiguousarray(
                    q_w[hs].transpose(2, 0, 1).reshape(D, HPC * P)),
                "kwT": np.ascontiguousarray(
                    k_w[hs].transpose(2, 0, 1).reshape(D, HPC * P)),
                "vwT": np.ascontiguousarray(
                    v_w[hs].transpose(2, 0, 1).reshape(D, HPC * P)),
                "qb": np.ascontiguousarray(q_b[hs].reshape(HPC * P, 1)),
                "kb": np.ascontiguousarray(k_b[hs].reshape(HPC * P, 1)),
                "vb": np.ascontiguousarray(v_b[hs].reshape(1, HPC * P)),
                "lwT": np.ascontiguousarray(l_w[:, f0:f1].T),
                "ones": ones,
            })
    return in_maps


def _run(inputs, trace=False):
    f32 = lambda a: np.asarray(a, dtype=np.float32)
    x = f32(inputs["x"])
    l_b = f32(inputs["l_b"])
    in_maps = _prep_in_maps(
        x, f32(inputs["q_w"]), f32(inputs["q_b"]), f32(inputs["k_w"]),
        f32(inputs["k_b"]), f32(inputs["v_w"]), f32(inputs["v_b"]),
        f32(inputs["l_w"]))
    nc = _get_nc()
    res = run_bass_kernel_spmd(nc, in_maps, list(range(N_CORES)), trace=trace)
    out = np.empty((B, S, NUM_OUT), dtype=np.float32)
    for b in range(B):
        acc = res.results[b * 4]["out"].astype(np.float32)
        for g in range(1, 4):
            acc = acc + res.results[b * 4 + g]["out"]
        out[b] = acc + l_b
    return out, res


def kernel(**inputs):
    out, _ = _run(inputs, trace=False)
    return out
